# revision 12
# baseline (speedup 1.0000x reference)
"""Trainium2 Bass kernel for the controlled-unitary problem.

reference semantics (control=0, num_qubits=13, dim=8192):
    mask bit = 1 << 12, so columns/rows with that bit set are idx 4096..8191.
    out[:, c0] = state[:, c0]                       (control bit off: untouched)
    out[:, c1] = state[:, c1] @ target[c1, c1]      (controlled unitary)

Device work: complex [256,4096] @ [4096,4096] GEMM, Gauss 3-mult form.
Sharding: output columns of the GEMM split 8 ways (each core gets a
[4096, 512] slab of the target block; every weight byte moves once).

Per-core kernel (v3):
  - ONE packed input dram tensor x[128, KT, 2, 768] holding all four
    planes (a_r|b_r / a_i|b_i interleaved per k-tile) -> one dma_start
    per chunk, 3KB-per-partition descriptors, rings alternated.
  - Gauss prep on views: Vector computes a_s and negates a_i in place;
    GpSimd computes b_s and b_d (in place over b_i).
  - k1 matmuls first per chunk so the PE starts on a_s/b_r while the
    B-side preps still run; last chunk is m-major so m0's epilogue
    hides under m1's matmuls.
  - Epilogue adds read the two PSUM banks directly (one tensor_tensor
    per output plane), Vector and GpSimd in parallel, outputs DMA'd
    per m-tile on both rings.
"""

import os

import numpy as np

BATCH = 256
DIM = 8192
HALF = 4096
N_CORES = 8
NSH = HALF // N_CORES  # 512 output columns per core
KT = HALF // 128  # 32 k-tiles
MT = BATCH // 128  # 2 m-tiles
FW = BATCH + NSH  # 768 packed free width per (ktile, re/im)
CHUNKS = [1, 1, 2, 2, 4, 4, 6, 6, 6]  # k-tiles per DMA chunk (sums to KT)

DT_NAME = os.environ.get("KERNEL_DT", "float16")

_CACHE = {}


def _np_dtype(dt_name):
    if dt_name == "float16":
        return np.float16
    if dt_name == "bfloat16":
        import ml_dtypes

        return ml_dtypes.bfloat16
    return np.float32


def _build(dt_name):
    import concourse.mybir as mybir
    import concourse.tile as tile
    from concourse import bacc

    DT = getattr(mybir.dt, dt_name)
    F32 = mybir.dt.float32

    nc = bacc.Bacc("TRN2", target_bir_lowering=False, debug=False,
                   num_devices=N_CORES)

    x = nc.dram_tensor("x", [128, KT, 2, FW], DT, kind="ExternalInput")
    c = nc.dram_tensor("c", [2, BATCH, NSH], F32, kind="ExternalOutput")

    with tile.TileContext(nc) as tc:
        with (
            tc.tile_pool(name="xp", bufs=2) as xp,
            tc.tile_pool(name="dp", bufs=2) as dp,
            tc.tile_pool(name="op", bufs=1) as op,
            tc.tile_pool(name="ps", bufs=1, space="PSUM") as ps_pool,
        ):
            # Gauss 3-multiplication complex GEMM:
            #   k1 = (a_r+a_i).b_r   k2 = a_r.(b_i-b_r)   k3 = (-a_i).(b_r+b_i)
            #   C_r = k1 + k3        C_i = k1 + k2
            ps = {}
            for m in range(MT):
                for comp in ("t1", "t2", "t3"):
                    ps[(m, comp)] = ps_pool.tile(
                        [128, NSH], F32, name=f"ps_{m}_{comp}"
                    )

            # three DMA queues: SP + ACT HWDGE rings, Pool SWDGE ring.
            # Each tops out well below HBM rate, so spread the load.
            rings = [nc.sync, nc.scalar, nc.gpsimd]

            # PE warm-up: the HAM throttle holds the PE at 1.2 GHz for its
            # first ~3.4us of busy time.  The PE would otherwise idle while
            # the first DMA lands, so burn that window on dummy matmuls.
            warm = op.tile([128, NSH], DT, name="warm")
            wps = ps_pool.tile([128, NSH], F32, name="ps_warm")
            nc.gpsimd.memset(warm[:], 0)
            for _ in range(8):
                nc.tensor.matmul(wps[:], warm[:, :128], warm[:],
                                 start=True, stop=True)

            k0 = 0
            for ci, ch in enumerate(CHUNKS):
                nb = 3 if ch == 6 else 2
                x_t = xp.tile([128, ch, 2, FW], DT, name=f"x{ch}", bufs=nb)
                rings[ci % 3].dma_start(x_t[:], x[:, k0:k0 + ch, :, :])

                as_t = dp.tile([128, ch, BATCH], DT, name=f"as{ch}", bufs=nb)
                bs_t = dp.tile([128, ch, NSH], DT, name=f"bs{ch}", bufs=nb)
                # prep per k-tile so every DVE op sees contiguous APs;
                # all on Vector (GpSimd's queue is busy driving DMAs)
                for kk in range(ch):
                    nc.vector.tensor_tensor(
                        as_t[:, kk, :], x_t[:, kk, 0, :BATCH],
                        x_t[:, kk, 1, :BATCH], mybir.AluOpType.add)
                    nc.vector.tensor_tensor(
                        bs_t[:, kk, :], x_t[:, kk, 0, BATCH:],
                        x_t[:, kk, 1, BATCH:], mybir.AluOpType.add)

                # t-scheme: t1 = Ar@Br, t2 = Ai@Bi, t3 = (Ar+Ai)@(Br+Bi)
                #   C_r = t1 - t2,  C_i = t3 - t1 - t2
                # t1/t2 read raw DMA data -> no prep on the PE critical path.
                def ops(comp, kk, msl):
                    if comp == "t1":
                        return x_t[:, kk, 0, msl.start:msl.stop], \
                            x_t[:, kk, 0, BATCH:]
                    if comp == "t2":
                        return x_t[:, kk, 1, msl.start:msl.stop], \
                            x_t[:, kk, 1, BATCH:]
                    return as_t[:, kk, msl], bs_t[:, kk, :]

                last_chunk = k0 + ch == KT
                if last_chunk:
                    # m-major: m0's epilogue hides under m1's matmuls
                    order = [(m, comp) for m in range(MT)
                             for comp in ("t1", "t2", "t3")]
                else:
                    order = [(m, comp) for comp in ("t1", "t2", "t3")
                             for m in range(MT)]
                for m, comp in order:
                    msl = slice(m * 128, (m + 1) * 128)
                    for kk in range(ch):
                        k = k0 + kk
                        lhs_v, rhs_v = ops(comp, kk, msl)
                        nc.tensor.matmul(
                            ps[(m, comp)][:], lhs_v, rhs_v, start=(k == 0),
                            stop=(last_chunk and kk == ch - 1),
                        )
                    if last_chunk and comp == "t1":
                        c1 = op.tile([128, NSH], F32, name=f"c1_{m}")
                        nc.scalar.activation(c1[:], ps[(m, "t1")][:],
                                             mybir.ActivationFunctionType.Copy)
                    if last_chunk and comp == "t2":
                        # out_r and u computed while t3 still streams
                        c2 = op.tile([128, NSH], F32, name=f"c2_{m}")
                        nc.scalar.activation(c2[:], ps[(m, "t2")][:],
                                             mybir.ActivationFunctionType.Copy)
                        out_r = op.tile([128, NSH], F32, name=f"or{m}")
                        u = op.tile([128, NSH], F32, name=f"u{m}")
                        nc.vector.tensor_tensor(
                            out_r[:], c1[:], c2[:], mybir.AluOpType.subtract)
                        nc.gpsimd.tensor_tensor(
                            u[:], c1[:], c2[:], mybir.AluOpType.add)
                        rings[m % 2].dma_start(c[0, msl, :], out_r[:])
                    if last_chunk and comp == "t3":
                        out_i = op.tile([128, NSH], F32, name=f"oi{m}")
                        nc.vector.tensor_tensor(
                            out_i[:], ps[(m, "t3")][:], u[:],
                            mybir.AluOpType.subtract)
                        rings[(m + 1) % 2].dma_start(c[1, msl, :], out_i[:])
                k0 += ch

    nc.compile()
    return nc


def _get_nc(dt_name):
    if dt_name not in _CACHE:
        _CACHE[dt_name] = _build(dt_name)
    return _CACHE[dt_name]


def _pack_inputs(A, B, np_dt):
    """A: [256, 4096] complex64, B: [4096, 4096] complex64 (full slab).
    Returns per-core packed x arrays [128, KT, 2, 768]."""
    at = A.T  # [4096, 256]
    # [4096, F] -> [128, KT, F] with k = kt*128 + p
    def kxm(m):
        f = m.shape[1]
        return m.reshape(KT, 128, f).transpose(1, 0, 2)

    a_r = kxm(np.ascontiguousarray(at.real))
    a_i = kxm(np.ascontiguousarray(at.imag))
    xs = []
    for cidx in range(N_CORES):
        csl = slice(cidx * NSH, (cidx + 1) * NSH)
        b_r = kxm(np.ascontiguousarray(B.real[:, csl]))
        b_i = kxm(np.ascontiguousarray(B.imag[:, csl]))
        xc = np.empty((128, KT, 2, FW), dtype=np_dt)
        xc[:, :, 0, :BATCH] = a_r
        xc[:, :, 0, BATCH:] = b_r
        xc[:, :, 1, :BATCH] = a_i
        xc[:, :, 1, BATCH:] = b_i
        xs.append(xc)
    return xs


def run_device(A, B, dt_name=DT_NAME, trace=False):
    """A: [256, 4096] complex64, B: [4096, 4096] complex64.
    Returns C = A @ B as [256, 4096] complex64 plus the raw results."""
    from concourse import bass_utils

    nc = _get_nc(dt_name)
    np_dt = _np_dtype(dt_name)

    xs = _pack_inputs(A, B, np_dt)
    in_maps = [{"x": xc} for xc in xs]

    res = bass_utils.run_bass_kernel_spmd(
        nc, in_maps, core_ids=list(range(N_CORES)), trace=trace
    )

    out = np.empty((BATCH, HALF), dtype=np.complex64)
    for cidx in range(N_CORES):
        csl = slice(cidx * NSH, (cidx + 1) * NSH)
        out.real[:, csl] = res.results[cidx]["c"][0]
        out.imag[:, csl] = res.results[cidx]["c"][1]
    return out, res


def kernel(state, target_matrix, control, num_qubits):
    state = np.asarray(state)
    target_matrix = np.asarray(target_matrix)
    control = int(control)
    num_qubits = int(num_qubits)
    dim = 1 << num_qubits

    assert state.shape == (BATCH, DIM) and dim == DIM, (
        "kernel hardcoded for [256, 8192]"
    )

    mask = 1 << (num_qubits - control - 1)
    idx = np.arange(dim)
    c1 = idx[(idx & mask) != 0]  # columns with control bit set

    if control == 0:
        A = state[:, HALF:]
        B = target_matrix[HALF:, HALF:]
    else:
        A = state[:, c1]
        B = target_matrix[np.ix_(c1, c1)]
    A = np.ascontiguousarray(A, dtype=np.complex64)
    B = np.ascontiguousarray(B, dtype=np.complex64)

    C, _ = run_device(A, B)

    out = state.astype(np.complex64, copy=True)
    out[:, c1] = C
    return out


# revision 13
# speedup vs baseline: 1.0250x; 1.0250x over previous
"""Trainium2 Bass kernel for the controlled-unitary problem.

reference semantics (control=0, num_qubits=13, dim=8192):
    mask bit = 1 << 12, so columns/rows with that bit set are idx 4096..8191.
    out[:, c0] = state[:, c0]                       (control bit off: untouched)
    out[:, c1] = state[:, c1] @ target[c1, c1]      (controlled unitary)

Device work: complex [256,4096] @ [4096,4096] GEMM.
Sharding: output columns of the GEMM split 8 ways (each core gets a
[4096, 512] slab of the target block; every weight byte moves once).

Per-core kernel (v5):
  - t-scheme complex GEMM: t1 = Ar@Br, t2 = Ai@Bi, t3 = (Ar+Ai)@(Br+Bi);
    C_r = t1 - t2, C_i = t3 - t1 - t2.  t1/t2 consume raw DMA data, so
    2/3 of the matmul stream has no DVE dependency at all.
  - THREE input dram tensors (A-pair, b_r, b_i; 4 MB each) DMA'd on
    three queues: SP + ACT HWDGE rings and the Pool SWDGE ring.  A
    single queue tops out ~130 GB/s; three together reach HBM rate.
  - PE warm-up matmuls burn the HAM cold-clock window while the first
    chunk is still in flight.
  - Per-k-tile DVE preps (contiguous APs only - strided multi-ktile
    DVE ops fall off the fast path).
  - Last chunk is m-major with t3 last: ACT copies t1/t2 (with PSUM
    read) while t3 streams; Vector finishes C_r/C_i; outputs leave on
    separate queues per m-tile.
"""

import os

import numpy as np

BATCH = 256
DIM = 8192
HALF = 4096
N_CORES = 8
NSH = HALF // N_CORES  # 512 output columns per core
KT = HALF // 128  # 32 k-tiles
MT = BATCH // 128  # 2 m-tiles
CHUNKS = [1, 1, 2, 4, 8, 8, 8]  # k-tiles per DMA chunk (sums to KT)

DT_NAME = os.environ.get("KERNEL_DT", "float16")

_CACHE = {}


def _np_dtype(dt_name):
    if dt_name == "float16":
        return np.float16
    if dt_name == "bfloat16":
        import ml_dtypes

        return ml_dtypes.bfloat16
    return np.float32


def _build(dt_name):
    import concourse.mybir as mybir
    import concourse.tile as tile
    from concourse import bacc

    DT = getattr(mybir.dt, dt_name)
    F32 = mybir.dt.float32
    COPY = mybir.ActivationFunctionType.Copy

    nc = bacc.Bacc("TRN2", target_bir_lowering=False, debug=False,
                   num_devices=N_CORES)

    xa = nc.dram_tensor("xa", [128, KT, 2, BATCH], DT, kind="ExternalInput")
    xbr = nc.dram_tensor("xbr", [128, KT, NSH], DT, kind="ExternalInput")
    xbi = nc.dram_tensor("xbi", [128, KT, NSH], DT, kind="ExternalInput")
    c = nc.dram_tensor("c", [2, BATCH, NSH], F32, kind="ExternalOutput")

    with tile.TileContext(nc) as tc:
        with (
            tc.tile_pool(name="xp", bufs=2) as xp,
            tc.tile_pool(name="dp", bufs=2) as dp,
            tc.tile_pool(name="op", bufs=1) as op,
            tc.tile_pool(name="ps", bufs=1, space="PSUM") as ps_pool,
        ):
            ps = {}
            for m in range(MT):
                for comp in ("t1", "t2", "t3"):
                    ps[(m, comp)] = ps_pool.tile(
                        [128, NSH], F32, name=f"ps_{m}_{comp}"
                    )

            rings = [nc.sync, nc.scalar, nc.gpsimd]

            # PE warm-up: the HAM throttle holds the PE at 1.2 GHz for its
            # first ~3.4us busy; the PE idles that long waiting for the
            # first chunk anyway, so spend it on dummy matmuls.
            warm = op.tile([128, NSH], DT, name="warm")
            wps = ps_pool.tile([128, NSH], F32, name="ps_warm")
            nc.vector.memset(warm[:], 0)
            for _ in range(8):
                nc.tensor.matmul(wps[:], warm[:, :128], warm[:],
                                 start=True, stop=True)

            k0 = 0
            for ci, ch in enumerate(CHUNKS):
                nb = 3 if ch == 8 else 2
                a_t = xp.tile([128, ch, 2, BATCH], DT, name=f"a{ch}", bufs=nb)
                br_t = xp.tile([128, ch, NSH], DT, name=f"br{ch}", bufs=nb)
                bi_t = xp.tile([128, ch, NSH], DT, name=f"bi{ch}", bufs=nb)
                ksl = slice(k0, k0 + ch)
                rings[0].dma_start(a_t[:], xa[:, ksl, :, :])
                rings[1].dma_start(br_t[:], xbr[:, ksl, :])
                rings[2].dma_start(bi_t[:], xbi[:, ksl, :])

                as_t = dp.tile([128, ch, BATCH], DT, name=f"as{ch}", bufs=nb)
                bs_t = dp.tile([128, ch, NSH], DT, name=f"bs{ch}", bufs=nb)
                # per-k-tile preps keep every AP contiguous
                for kk in range(ch):
                    nc.vector.tensor_tensor(
                        as_t[:, kk, :], a_t[:, kk, 0, :],
                        a_t[:, kk, 1, :], mybir.AluOpType.add)
                    nc.vector.tensor_tensor(
                        bs_t[:, kk, :], br_t[:, kk, :],
                        bi_t[:, kk, :], mybir.AluOpType.add)

                def ops(comp, kk, msl):
                    if comp == "t1":
                        return a_t[:, kk, 0, msl.start:msl.stop], br_t[:, kk, :]
                    if comp == "t2":
                        return a_t[:, kk, 1, msl.start:msl.stop], bi_t[:, kk, :]
                    return as_t[:, kk, msl], bs_t[:, kk, :]

                last_chunk = k0 + ch == KT
                if last_chunk:
                    order = [(m, comp) for m in range(MT)
                             for comp in ("t1", "t2", "t3")]
                else:
                    order = [(m, comp) for comp in ("t1", "t2", "t3")
                             for m in range(MT)]
                for m, comp in order:
                    msl = slice(m * 128, (m + 1) * 128)
                    for kk in range(ch):
                        k = k0 + kk
                        lhs_v, rhs_v = ops(comp, kk, msl)
                        nc.tensor.matmul(
                            ps[(m, comp)][:], lhs_v, rhs_v, start=(k == 0),
                            stop=(last_chunk and kk == ch - 1),
                        )
                    if last_chunk and comp == "t1":
                        c1 = op.tile([128, NSH], F32, name=f"c1_{m}")
                        nc.scalar.activation(c1[:], ps[(m, "t1")][:], COPY)
                    if last_chunk and comp == "t2":
                        c2 = op.tile([128, NSH], F32, name=f"c2_{m}")
                        nc.scalar.activation(c2[:], ps[(m, "t2")][:], COPY)
                        out_r = op.tile([128, NSH], F32, name=f"or{m}")
                        u = op.tile([128, NSH], F32, name=f"u{m}")
                        nc.vector.tensor_tensor(
                            out_r[:], c1[:], c2[:], mybir.AluOpType.subtract)
                        nc.gpsimd.tensor_tensor(
                            u[:], c1[:], c2[:], mybir.AluOpType.add)
                        rings[m].dma_start(c[0, msl, :], out_r[:])
                    if last_chunk and comp == "t3":
                        out_i = op.tile([128, NSH], F32, name=f"oi{m}")
                        nc.vector.tensor_tensor(
                            out_i[:], ps[(m, "t3")][:], u[:],
                            mybir.AluOpType.subtract)
                        rings[2 - m].dma_start(c[1, msl, :], out_i[:])
                k0 += ch

    nc.compile()
    return nc


def _get_nc(dt_name):
    if dt_name not in _CACHE:
        _CACHE[dt_name] = _build(dt_name)
    return _CACHE[dt_name]


def _kxm(m):
    # [4096, F] -> [128, KT, F] with k = kt*128 + p
    f = m.shape[1]
    return m.reshape(KT, 128, f).transpose(1, 0, 2)


def _pack_inputs(A, B, np_dt):
    at = A.T  # [4096, 256]
    a_r = _kxm(np.ascontiguousarray(at.real))
    a_i = _kxm(np.ascontiguousarray(at.imag))
    xa = np.empty((128, KT, 2, BATCH), dtype=np_dt)
    xa[:, :, 0, :] = a_r
    xa[:, :, 1, :] = a_i
    maps = []
    for cidx in range(N_CORES):
        csl = slice(cidx * NSH, (cidx + 1) * NSH)
        maps.append({
            "xa": xa,
            "xbr": np.ascontiguousarray(
                _kxm(np.ascontiguousarray(B.real[:, csl]))).astype(np_dt),
            "xbi": np.ascontiguousarray(
                _kxm(np.ascontiguousarray(B.imag[:, csl]))).astype(np_dt),
        })
    return maps


def run_device(A, B, dt_name=DT_NAME, trace=False):
    """A: [256, 4096] complex64, B: [4096, 4096] complex64.
    Returns C = A @ B as [256, 4096] complex64 plus the raw results."""
    from concourse import bass_utils

    nc = _get_nc(dt_name)
    np_dt = _np_dtype(dt_name)

    in_maps = _pack_inputs(A, B, np_dt)
    res = bass_utils.run_bass_kernel_spmd(
        nc, in_maps, core_ids=list(range(N_CORES)), trace=trace
    )

    out = np.empty((BATCH, HALF), dtype=np.complex64)
    for cidx in range(N_CORES):
        csl = slice(cidx * NSH, (cidx + 1) * NSH)
        out.real[:, csl] = res.results[cidx]["c"][0]
        out.imag[:, csl] = res.results[cidx]["c"][1]
    return out, res


def kernel(state, target_matrix, control, num_qubits):
    state = np.asarray(state)
    target_matrix = np.asarray(target_matrix)
    control = int(control)
    num_qubits = int(num_qubits)
    dim = 1 << num_qubits

    assert state.shape == (BATCH, DIM) and dim == DIM, (
        "kernel hardcoded for [256, 8192]"
    )

    mask = 1 << (num_qubits - control - 1)
    idx = np.arange(dim)
    c1 = idx[(idx & mask) != 0]  # columns with control bit set

    if control == 0:
        A = state[:, HALF:]
        B = target_matrix[HALF:, HALF:]
    else:
        A = state[:, c1]
        B = target_matrix[np.ix_(c1, c1)]
    A = np.ascontiguousarray(A, dtype=np.complex64)
    B = np.ascontiguousarray(B, dtype=np.complex64)

    C, _ = run_device(A, B)

    out = state.astype(np.complex64, copy=True)
    out[:, c1] = C
    return out


# revision 15
# speedup vs baseline: 1.1247x; 1.0973x over previous
"""Trainium2 Bass kernel for the controlled-unitary problem.

reference semantics (control=0, num_qubits=13, dim=8192):
    mask bit = 1 << 12, so columns/rows with that bit set are idx 4096..8191.
    out[:, c0] = state[:, c0]                       (control bit off: untouched)
    out[:, c1] = state[:, c1] @ target[c1, c1]      (controlled unitary)

Device work: complex [256,4096] @ [4096,4096] GEMM.
Sharding: output columns of the GEMM split 8 ways (each core gets a
[4096, 512] slab of the target block; every weight byte moves once).

Per-core kernel (v6):
  - t-scheme complex GEMM: t1 = Ar@Br, t2 = Ai@Bi, t3 = (Ar+Ai)@(Br+Bi);
    C_r = t1 - t2, C_i = t3 - t1 - t2.  t1/t2 consume raw DMA bytes, so
    2/3 of the matmul stream has no DVE dependency.
  - Inputs stream over both HWDGE rings (each tops out ~130 GB/s) in
    v2-style per-plane chunk transfers; the LAST chunk's B planes ride
    the Pool SWDGE queue, issued up front - its slow completion path is
    hidden by ~28us of lead time, and it takes 2 MB off the rings.
  - PE warm-up matmuls burn the HAM cold-clock window while the first
    chunk is in flight.
  - Per-k-tile DVE preps (contiguous APs only).
  - Last chunk m-major with t3 last: ACT copies t1/t2 out of PSUM while
    t3 streams; Vector/GpSimd finish C_r/C_i; per-m outputs leave on
    both rings as soon as they're ready.
"""

import os

import numpy as np

BATCH = 256
DIM = 8192
HALF = 4096
N_CORES = 8
NSH = HALF // N_CORES  # 512 output columns per core
KT = HALF // 128  # 32 k-tiles
MT = BATCH // 128  # 2 m-tiles
CHUNKS = [1, 1, 2, 4, 8, 8, 8]  # k-tiles per chunk (sums to KT)

DT_NAME = os.environ.get("KERNEL_DT", "float16")

_CACHE = {}


def _np_dtype(dt_name):
    if dt_name == "float16":
        return np.float16
    if dt_name == "bfloat16":
        import ml_dtypes

        return ml_dtypes.bfloat16
    return np.float32


def _build(dt_name):
    import concourse.mybir as mybir
    import concourse.tile as tile
    from concourse import bacc

    DT = getattr(mybir.dt, dt_name)
    F32 = mybir.dt.float32
    COPY = mybir.ActivationFunctionType.Copy

    nc = bacc.Bacc("TRN2", target_bir_lowering=False, debug=False,
                   num_devices=N_CORES)

    xa = nc.dram_tensor("xa", [128, KT, 2, BATCH], DT, kind="ExternalInput")
    xbr = nc.dram_tensor("xbr", [128, KT, NSH], DT, kind="ExternalInput")
    xbi = nc.dram_tensor("xbi", [128, KT, NSH], DT, kind="ExternalInput")
    c = nc.dram_tensor("c", [2, BATCH, NSH], F32, kind="ExternalOutput")

    LAST = len(CHUNKS) - 1
    LK = CHUNKS[LAST]  # ktiles in the prefetched last chunk
    LK0 = KT - LK

    with tile.TileContext(nc) as tc:
        with (
            tc.tile_pool(name="xp", bufs=2) as xp,
            tc.tile_pool(name="dp", bufs=2) as dp,
            tc.tile_pool(name="op", bufs=1) as op,
            tc.tile_pool(name="ps", bufs=1, space="PSUM") as ps_pool,
        ):
            ps = {}
            for m in range(MT):
                for comp in ("t1", "t2", "t3"):
                    ps[(m, comp)] = ps_pool.tile(
                        [128, NSH], F32, name=f"ps_{m}_{comp}"
                    )

            rings = [nc.sync, nc.scalar]

            # SWDGE prefetch of the final chunk's B planes: consumed only
            # ~30us in, so the Pool queue's slow completion path is hidden,
            # and the HWDGE rings shed 2 MB.
            b8r_t = op.tile([128, LK, NSH], DT, name="b8r")
            b8i_t = op.tile([128, LK, NSH], DT, name="b8i")
            nc.gpsimd.dma_start(b8r_t[:], xbr[:, LK0:KT, :])
            nc.gpsimd.dma_start(b8i_t[:], xbi[:, LK0:KT, :])

            # PE warm-up: HAM holds the PE at 1.2 GHz for its first ~3.4us
            # of busy time; the PE idles that long waiting for chunk 0
            # anyway, so burn the window on dummy matmuls.
            warm = op.tile([128, NSH], DT, name="warm")
            wps = ps_pool.tile([128, NSH], F32, name="ps_warm")
            nc.vector.memset(warm[:], 0)
            for _ in range(8):
                nc.tensor.matmul(wps[:], warm[:, :128], warm[:],
                                 start=True, stop=True)

            k0 = 0
            for ci, ch in enumerate(CHUNKS):
                nb = 2
                last_chunk = ci == LAST
                a_t = xp.tile([128, ch, 2, BATCH], DT, name=f"a{ch}", bufs=nb)
                ksl = slice(k0, k0 + ch)
                rings[ci % 2].dma_start(a_t[:], xa[:, ksl, :, :])
                if last_chunk:
                    br_t, bi_t = b8r_t, b8i_t
                else:
                    br_t = xp.tile([128, ch, NSH], DT, name=f"br{ch}", bufs=nb)
                    bi_t = xp.tile([128, ch, NSH], DT, name=f"bi{ch}", bufs=nb)
                    rings[(ci + 1) % 2].dma_start(br_t[:], xbr[:, ksl, :])
                    rings[ci % 2].dma_start(bi_t[:], xbi[:, ksl, :])

                as_t = dp.tile([128, ch, BATCH], DT, name=f"as{ch}", bufs=nb)
                bs_t = dp.tile([128, ch, NSH], DT, name=f"bs{ch}", bufs=nb)
                # per-k-tile preps keep every AP contiguous
                for kk in range(ch):
                    nc.vector.tensor_tensor(
                        as_t[:, kk, :], a_t[:, kk, 0, :],
                        a_t[:, kk, 1, :], mybir.AluOpType.add)
                    nc.vector.tensor_tensor(
                        bs_t[:, kk, :], br_t[:, kk, :],
                        bi_t[:, kk, :], mybir.AluOpType.add)

                def ops(comp, kk, msl):
                    if comp == "t1":
                        return a_t[:, kk, 0, msl.start:msl.stop], br_t[:, kk, :]
                    if comp == "t2":
                        return a_t[:, kk, 1, msl.start:msl.stop], bi_t[:, kk, :]
                    return as_t[:, kk, msl], bs_t[:, kk, :]

                if last_chunk:
                    order = [(m, comp) for m in range(MT)
                             for comp in ("t1", "t2", "t3")]
                else:
                    order = [(m, comp) for comp in ("t1", "t2", "t3")
                             for m in range(MT)]
                for m, comp in order:
                    msl = slice(m * 128, (m + 1) * 128)
                    for kk in range(ch):
                        k = k0 + kk
                        lhs_v, rhs_v = ops(comp, kk, msl)
                        nc.tensor.matmul(
                            ps[(m, comp)][:], lhs_v, rhs_v, start=(k == 0),
                            stop=(last_chunk and kk == ch - 1),
                        )
                    if last_chunk and comp == "t1":
                        c1 = op.tile([128, NSH], F32, name=f"c1_{m}")
                        nc.scalar.activation(c1[:], ps[(m, "t1")][:], COPY)
                    if last_chunk and comp == "t2":
                        c2 = op.tile([128, NSH], F32, name=f"c2_{m}")
                        nc.scalar.activation(c2[:], ps[(m, "t2")][:], COPY)
                        out_r = op.tile([128, NSH], F32, name=f"or{m}")
                        u = op.tile([128, NSH], F32, name=f"u{m}")
                        nc.vector.tensor_tensor(
                            out_r[:], c1[:], c2[:], mybir.AluOpType.subtract)
                        nc.gpsimd.tensor_tensor(
                            u[:], c1[:], c2[:], mybir.AluOpType.add)
                        rings[m % 2].dma_start(c[0, msl, :], out_r[:])
                    if last_chunk and comp == "t3":
                        out_i = op.tile([128, NSH], F32, name=f"oi{m}")
                        nc.vector.tensor_tensor(
                            out_i[:], ps[(m, "t3")][:], u[:],
                            mybir.AluOpType.subtract)
                        rings[(m + 1) % 2].dma_start(c[1, msl, :], out_i[:])
                k0 += ch

    nc.compile()
    return nc


def _get_nc(dt_name):
    if dt_name not in _CACHE:
        _CACHE[dt_name] = _build(dt_name)
    return _CACHE[dt_name]


def _kxm(m):
    # [4096, F] -> [128, KT, F] with k = kt*128 + p
    f = m.shape[1]
    return m.reshape(KT, 128, f).transpose(1, 0, 2)


def _pack_inputs(A, B, np_dt):
    at = A.T  # [4096, 256]
    a_r = _kxm(np.ascontiguousarray(at.real))
    a_i = _kxm(np.ascontiguousarray(at.imag))
    xa = np.empty((128, KT, 2, BATCH), dtype=np_dt)
    xa[:, :, 0, :] = a_r
    xa[:, :, 1, :] = a_i
    maps = []
    for cidx in range(N_CORES):
        csl = slice(cidx * NSH, (cidx + 1) * NSH)
        maps.append({
            "xa": xa,
            "xbr": np.ascontiguousarray(
                _kxm(np.ascontiguousarray(B.real[:, csl]))).astype(np_dt),
            "xbi": np.ascontiguousarray(
                _kxm(np.ascontiguousarray(B.imag[:, csl]))).astype(np_dt),
        })
    return maps


def run_device(A, B, dt_name=DT_NAME, trace=False):
    """A: [256, 4096] complex64, B: [4096, 4096] complex64.
    Returns C = A @ B as [256, 4096] complex64 plus the raw results."""
    from concourse import bass_utils

    nc = _get_nc(dt_name)
    np_dt = _np_dtype(dt_name)

    in_maps = _pack_inputs(A, B, np_dt)
    res = bass_utils.run_bass_kernel_spmd(
        nc, in_maps, core_ids=list(range(N_CORES)), trace=trace
    )

    out = np.empty((BATCH, HALF), dtype=np.complex64)
    for cidx in range(N_CORES):
        csl = slice(cidx * NSH, (cidx + 1) * NSH)
        out.real[:, csl] = res.results[cidx]["c"][0]
        out.imag[:, csl] = res.results[cidx]["c"][1]
    return out, res


def kernel(state, target_matrix, control, num_qubits):
    state = np.asarray(state)
    target_matrix = np.asarray(target_matrix)
    control = int(control)
    num_qubits = int(num_qubits)
    dim = 1 << num_qubits

    assert state.shape == (BATCH, DIM) and dim == DIM, (
        "kernel hardcoded for [256, 8192]"
    )

    mask = 1 << (num_qubits - control - 1)
    idx = np.arange(dim)
    c1 = idx[(idx & mask) != 0]  # columns with control bit set

    if control == 0:
        A = state[:, HALF:]
        B = target_matrix[HALF:, HALF:]
    else:
        A = state[:, c1]
        B = target_matrix[np.ix_(c1, c1)]
    A = np.ascontiguousarray(A, dtype=np.complex64)
    B = np.ascontiguousarray(B, dtype=np.complex64)

    C, _ = run_device(A, B)

    out = state.astype(np.complex64, copy=True)
    out[:, c1] = C
    return out
